# revision 1
# baseline (speedup 1.0000x reference)
"""Self-contained Trainium2 Bass kernel for nn_Attention_51840255263121.

Full attention block: QKV projection + QK-RMSNorm + RoPE (rotate-half) +
non-causal SDPA + output projection, for B=2, N=2048, C=2048, H=16, D=128.

Sharding: 8 NeuronCores over (batch, head-group): core = b*4 + hg owns batch b
and heads hg*4..hg*4+3 (512 channels). Each core computes its heads' attention
output and a partial output projection over its 512 channels; the host sums the
4 partials per batch and adds the bias.

All matmuls run as float32r (TF32-like, ~1.5e-4 rel err, full PE rate).
"""

import numpy as np

B, N, C, H, D = 2, 2048, 2048, 16, 128
NCORES = 8
HPC = 4          # heads per core
CS = HPC * D     # 512 channels per core
NT = N // 128    # 16 n-tiles
CT = C // 128    # 16 c-tiles
EPS = 1e-6
NCHUNK = 256     # stage-A xT n-chunk
NQC = 512        # stage-B nq chunk
SPLIT_QK_LOADS = True
SB_BUFS = (3, 2, 2)  # (scoresT, pv, sums) PSUM pool bufs


def build_nc():
    import concourse.bacc as bacc
    import concourse.mybir as mybir
    import concourse.tile as tile
    from concourse.masks import make_identity

    F32 = mybir.dt.float32
    F32R = mybir.dt.float32r
    AF = mybir.ActivationFunctionType
    ALU = mybir.AluOpType

    nc = bacc.Bacc(None, target_bir_lowering=False, debug=False)

    xT = nc.declare_dram_parameter("xT", [C, N], F32R, isOutput=False)
    wT = nc.declare_dram_parameter("wT", [C, 3 * CS], F32R, isOutput=False)
    pwT = nc.declare_dram_parameter("pwT", [CS, C], F32R, isOutput=False)
    cosq = nc.declare_dram_parameter("cosq", [N, D], F32, isOutput=False)
    sinq = nc.declare_dram_parameter("sinq", [N, D], F32, isOutput=False)
    cosk = nc.declare_dram_parameter("cosk", [N, D], F32, isOutput=False)
    sink = nc.declare_dram_parameter("sink", [N, D], F32, isOutput=False)
    ones = nc.declare_dram_parameter("ones", [128, 1], F32R, isOutput=False)
    outp = nc.declare_dram_parameter("outp", [N, C], F32, isOutput=True)

    # staging for qT/kT: 8 planes of [128, N] (q heads 0..3 then k heads 0..3)
    qkT = nc.dram_tensor("qkT_stage", [128, 8 * N], F32R)

    with tile.TileContext(nc) as tc:
        import contextlib

        with contextlib.ExitStack() as octx:
            # pools that live across stages
            persist = octx.enter_context(tc.tile_pool(name="persist", bufs=1))
            v_sb = [persist.tile([128, CS], F32R, name=f"v{i}") for i in range(NT)]
            ident = persist.tile([128, 128], F32, name="ident")
            make_identity(nc, ident[:])
            ones_sb = persist.tile([128, 1], F32R, name="ones_sb")
            nc.sync.dma_start(out=ones_sb[:], in_=ones[:, :])
            eps_sb = persist.tile([128, 1], F32, name="eps_sb")
            nc.vector.memset(eps_sb[:], EPS)

            # ---------------- Stage A: QKV + rmsnorm + rope + transpose ----
            with contextlib.ExitStack() as actx:
                p_wt = actx.enter_context(tc.tile_pool(name="p_wt", bufs=1))
                p_xt = actx.enter_context(tc.tile_pool(name="p_xt", bufs=2))
                p_cs = actx.enter_context(tc.tile_pool(name="p_cs", bufs=2))
                p_ps = actx.enter_context(tc.tile_pool(name="p_ps", bufs=2, space="PSUM"))
                p_pst = actx.enter_context(tc.tile_pool(name="p_pst", bufs=1, space="PSUM"))
                p_sc = actx.enter_context(tc.tile_pool(name="p_sc", bufs=2))
                p_ro = actx.enter_context(tc.tile_pool(name="p_ro", bufs=2))
                p_ev = actx.enter_context(tc.tile_pool(name="p_ev", bufs=2))

                wt_sb = [p_wt.tile([128, 3 * CS], F32R, name=f"wt{i}") for i in range(CT)]
                xt0_sb = [p_xt.tile([128, NCHUNK], F32R, name=f"xt{i}") for i in range(CT)]
                for i in range(CT):
                    nc.sync.dma_start(out=xt0_sb[i][:], in_=xT[i * 128:(i + 1) * 128, 0:NCHUNK])
                    nc.sync.dma_start(out=wt_sb[i][:], in_=wT[i * 128:(i + 1) * 128, :])

                def emit_transposes(nt_p, ro_list_p):
                    psT_q = p_pst.tile([128, CS], F32, name="psT_q")
                    psT_k = p_pst.tile([128, CS], F32, name="psT_k")
                    for t, psT in ((0, psT_q), (1, psT_k)):
                        for hl in range(HPC):
                            nc.tensor.transpose(psT[:, hl * D:(hl + 1) * D],
                                                ro_list_p[t * 4 + hl][:], ident[:])
                    bq_t = p_ro.tile([128, CS], F32R, name="bq_t")
                    bk_t = p_ro.tile([128, CS], F32R, name="bk_t")
                    nc.scalar.copy(bq_t[:], psT_q[:])
                    nc.scalar.copy(bk_t[:], psT_k[:])
                    for hl in range(HPC):
                        nc.sync.dma_start(
                            out=qkT[:, hl * N + nt_p * 128:hl * N + (nt_p + 1) * 128],
                            in_=bq_t[:, hl * D:(hl + 1) * D])
                        nc.sync.dma_start(
                            out=qkT[:, (4 + hl) * N + nt_p * 128:(4 + hl) * N + (nt_p + 1) * 128],
                            in_=bk_t[:, hl * D:(hl + 1) * D])

                for ch in range(N // NCHUNK):
                    n0 = ch * NCHUNK
                    if ch == 0:
                        xt_sb = xt0_sb
                    else:
                        xt_sb = [p_xt.tile([128, NCHUNK], F32R, name=f"xt{i}") for i in range(CT)]
                        for i in range(CT):
                            nc.sync.dma_start(
                                out=xt_sb[i][:],
                                in_=xT[i * 128:(i + 1) * 128, n0:n0 + NCHUNK])
                    for sub in range(NCHUNK // 128):
                        nt = (n0 + sub * 128) // 128
                        nsl = slice(nt * 128, (nt + 1) * 128)
                        cq_t = p_cs.tile([128, D], F32, name="cq_t")
                        sq_t = p_cs.tile([128, D], F32, name="sq_t")
                        ck_t = p_cs.tile([128, D], F32, name="ck_t")
                        sk_t = p_cs.tile([128, D], F32, name="sk_t")
                        nc.sync.dma_start(out=cq_t[:], in_=cosq[nsl, :])
                        nc.sync.dma_start(out=sq_t[:], in_=sinq[nsl, :])
                        nc.sync.dma_start(out=ck_t[:], in_=cosk[nsl, :])
                        nc.sync.dma_start(out=sk_t[:], in_=sink[nsl, :])

                        ps_q = p_ps.tile([128, CS], F32, name="ps_q")
                        ps_k = p_ps.tile([128, CS], F32, name="ps_k")
                        ps_v = p_ps.tile([128, CS], F32, name="ps_v")
                        for ci in range(CT):
                            st, sp = (ci == 0), (ci == CT - 1)
                            lhs = xt_sb[ci][:, sub * 128:(sub + 1) * 128]
                            nc.tensor.matmul(ps_q[:], lhs, wt_sb[ci][:, 0:CS],
                                             start=st, stop=sp)
                            nc.tensor.matmul(ps_k[:], lhs, wt_sb[ci][:, CS:2 * CS],
                                             start=st, stop=sp)
                            nc.tensor.matmul(ps_v[:], lhs, wt_sb[ci][:, 2 * CS:3 * CS],
                                             start=st, stop=sp)

                        # evac q,k,v psums to SBUF right away (frees banks;
                        # SBUF-src DVE ops are faster than PSUM-src)
                        q_sb = p_ev.tile([128, CS], F32, name="q_sb")
                        k_sb = p_ev.tile([128, CS], F32, name="k_sb")
                        nc.scalar.copy(q_sb[:], ps_q[:])
                        nc.scalar.copy(k_sb[:], ps_k[:])
                        nc.vector.tensor_copy(v_sb[nt][:], ps_v[:])

                        # rmsnorm stats for q,k (8 head-slices):
                        # ACT Square + accum_out -> per-row sum of squares
                        stats = p_sc.tile([128, 8], F32, name="stats")
                        dump = p_sc.tile([128, 128], F32, name="dump")
                        for t, ps in ((0, q_sb), (1, k_sb)):
                            for hl in range(HPC):
                                nc.scalar.activation(
                                    dump[:], ps[:, hl * D:(hl + 1) * D], AF.Square,
                                    accum_out=stats[:, t * 4 + hl:t * 4 + hl + 1])
                        # rstat = 1/sqrt(sumsq/D + eps)
                        rstat = p_sc.tile([128, 8], F32, name="rstat")
                        nc.scalar.activation(rstat[:], stats[:], AF.Sqrt,
                                             bias=eps_sb[:], scale=1.0 / D)
                        nc.vector.reciprocal(rstat[:], rstat[:])

                        # rope per head-slice, then transpose
                        ro_list = []
                        for t, ps, cos_t, sin_t in (
                                (0, q_sb, cq_t, sq_t),
                                (1, k_sb, ck_t, sk_t)):
                            for hl in range(HPC):
                                hsl = slice(hl * D, (hl + 1) * D)
                                r = rstat[:, t * 4 + hl:t * 4 + hl + 1]
                                tc_t = p_ro.tile([128, D], F32, name="tc_t")
                                ts_t = p_ro.tile([128, D], F32, name="ts_t")
                                ro_t = p_ro.tile([128, D], F32, name="ro_t")
                                nc.vector.scalar_tensor_tensor(
                                    out=tc_t[:], in0=ps[:, hsl], scalar=r,
                                    in1=cos_t[:], op0=ALU.mult, op1=ALU.mult)
                                nc.vector.scalar_tensor_tensor(
                                    out=ts_t[:, 0:64],
                                    in0=ps[:, hl * D + 64:hl * D + 128], scalar=r,
                                    in1=sin_t[:, 0:64], op0=ALU.mult, op1=ALU.mult)
                                nc.vector.scalar_tensor_tensor(
                                    out=ts_t[:, 64:128],
                                    in0=ps[:, hl * D:hl * D + 64], scalar=r,
                                    in1=sin_t[:, 64:128], op0=ALU.mult, op1=ALU.mult)
                                nc.vector.tensor_add(ro_t[:], tc_t[:], ts_t[:])
                                ro_list.append(ro_t)
                        emit_transposes(nt, ro_list)

            # ---------------- Stage B: attention per head --------------------
            p_bc = octx.enter_context(tc.tile_pool(name="p_bc", bufs=1))
            outT = [p_bc.tile([128, N], F32R, name=f"outT{h}") for h in range(HPC)]
            pwT_sb = [p_bc.tile([128, C], F32R, name=f"pw{h}") for h in range(HPC)]
            for h in range(HPC):
                nc.sync.dma_start(out=pwT_sb[h][:], in_=pwT[h * 128:(h + 1) * 128, :])

            with contextlib.ExitStack() as bctx:
                p_qk = bctx.enter_context(tc.tile_pool(name="p_qk", bufs=2))
                p_sT = bctx.enter_context(tc.tile_pool(name="p_sT", bufs=SB_BUFS[0], space="PSUM"))
                p_pv = bctx.enter_context(tc.tile_pool(name="p_pv", bufs=SB_BUFS[1], space="PSUM"))
                p_sm = bctx.enter_context(tc.tile_pool(name="p_sm", bufs=SB_BUFS[2], space="PSUM"))
                p_pt = bctx.enter_context(tc.tile_pool(name="p_pt", bufs=6))
                p_sb = bctx.enter_context(tc.tile_pool(name="p_sb", bufs=2))

                for h in range(HPC):
                    qT_h = p_qk.tile([128, N], F32R, name="qT_h")
                    kT_h = p_qk.tile([128, N], F32R, name="kT_h")
                    if SPLIT_QK_LOADS:
                        for q4 in range(4):
                            qsl4 = slice(q4 * 512, (q4 + 1) * 512)
                            nc.sync.dma_start(out=qT_h[:, qsl4],
                                              in_=qkT[:, h * N + q4 * 512:h * N + (q4 + 1) * 512])
                            nc.sync.dma_start(out=kT_h[:, qsl4],
                                              in_=qkT[:, (4 + h) * N + q4 * 512:(4 + h) * N + (q4 + 1) * 512])
                    else:
                        nc.sync.dma_start(out=qT_h[:], in_=qkT[:, h * N:(h + 1) * N])
                        nc.sync.dma_start(out=kT_h[:], in_=qkT[:, (4 + h) * N:(5 + h) * N])

                    for cq in range(N // NQC):
                        q0 = cq * NQC
                        pv_ps = p_pv.tile([128, NQC], F32, name="pv_ps")
                        sm_ps = p_sm.tile([1, NQC], F32, name="sm_ps")
                        for nk in range(NT):
                            ksl = slice(nk * 128, (nk + 1) * 128)
                            sT_ps = p_sT.tile([128, NQC], F32, name="sT_ps")
                            pt_t = p_pt.tile([128, NQC], F32R, name="pt_t")
                            for half in range(NQC // 512):
                                fsl = slice(half * 512, (half + 1) * 512)
                                qsl = slice(q0 + half * 512, q0 + (half + 1) * 512)
                                nc.tensor.matmul(sT_ps[:, fsl], kT_h[:, ksl],
                                                 qT_h[:, qsl], start=True, stop=True)
                            nc.scalar.activation(pt_t[:], sT_ps[:], AF.Exp)
                            st, sp = (nk == 0), (nk == NT - 1)
                            for half in range(NQC // 512):
                                fsl = slice(half * 512, (half + 1) * 512)
                                nc.tensor.matmul(pv_ps[:, fsl],
                                                 v_sb[nk][:, h * D:(h + 1) * D],
                                                 pt_t[:, fsl], start=st, stop=sp)
                                nc.tensor.matmul(sm_ps[:, fsl], ones_sb[:],
                                                 pt_t[:, fsl], start=st, stop=sp)

                        sums = p_sb.tile([1, NQC], F32, name="sums")
                        recip = p_sb.tile([1, NQC], F32, name="recip")
                        bcast = p_sb.tile([128, NQC], F32, name="bcast")
                        nc.scalar.copy(sums[:], sm_ps[:])
                        nc.vector.reciprocal(recip[:], sums[:])
                        nc.gpsimd.partition_broadcast(bcast[:], recip[:])
                        nc.vector.tensor_mul(outT[h][:, q0:q0 + NQC], pv_ps[:], bcast[:])

            # ---------------- Stage C: output projection ---------------------
            with contextlib.ExitStack() as cctx:
                p_pc = cctx.enter_context(tc.tile_pool(name="p_pc", bufs=4, space="PSUM"))
                p_fo = cctx.enter_context(tc.tile_pool(name="p_fo", bufs=4))
                for nt in range(NT):
                    for oc in range(C // 512):
                        ps_c = p_pc.tile([128, 512], F32, name="ps_c")
                        for h in range(HPC):
                            nc.tensor.matmul(ps_c[:],
                                             outT[h][:, nt * 128:(nt + 1) * 128],
                                             pwT_sb[h][:, oc * 512:(oc + 1) * 512],
                                             start=(h == 0), stop=(h == HPC - 1))
                        fo_t = p_fo.tile([128, 512], F32, name="fo_t")
                        nc.vector.tensor_copy(fo_t[:], ps_c[:])
                        nc.sync.dma_start(
                            out=outp[nt * 128:(nt + 1) * 128, oc * 512:(oc + 1) * 512],
                            in_=fo_t[:])

    nc.finalize()
    return nc


def make_in_maps(x, rope_cos, rope_sin, qkv_w, proj_w, q_norm_w, k_norm_w):
    scale = np.float32(D ** -0.5)

    def fold(w, scaled):
        cos = rope_cos * w[None, :]
        sf = np.empty_like(rope_sin)
        sf[:, :64] = -rope_sin[:, :64] * w[None, 64:]
        sf[:, 64:] = rope_sin[:, 64:] * w[None, :64]
        if scaled:
            cos = cos * scale
            sf = sf * scale
        return np.ascontiguousarray(cos, np.float32), np.ascontiguousarray(sf, np.float32)

    cosq, sinq = fold(q_norm_w, True)
    cosk, sink = fold(k_norm_w, False)
    ones = np.ones((128, 1), np.float32)

    in_maps = []
    for core in range(NCORES):
        b, hg = core // 4, core % 4
        c0 = hg * CS
        rows = np.concatenate([
            qkv_w[c0:c0 + CS], qkv_w[C + c0:C + c0 + CS],
            qkv_w[2 * C + c0:2 * C + c0 + CS]], axis=0)
        in_maps.append({
            "xT": np.ascontiguousarray(x[b].T, np.float32),
            "wT": np.ascontiguousarray(rows.T, np.float32),
            "pwT": np.ascontiguousarray(proj_w[:, c0:c0 + CS].T, np.float32),
            "cosq": cosq, "sinq": sinq, "cosk": cosk, "sink": sink,
            "ones": ones,
        })
    return in_maps


def gather(results, proj_b):
    out = np.empty((B, N, C), np.float32)
    for b in range(B):
        acc = np.zeros((N, C), np.float64)
        for hg in range(4):
            acc += results[b * 4 + hg]["outp"].astype(np.float64)
        out[b] = (acc + proj_b.astype(np.float64)[None, :]).astype(np.float32)
    return out


LAST_RESULTS = None  # BassKernelResults of the most recent kernel() call


def kernel(x, rope_cos, rope_sin, qkv_w, proj_w, proj_b, q_norm_w, k_norm_w):
    import os
    from concourse.bass_utils import run_bass_kernel_spmd

    global LAST_RESULTS
    x = np.asarray(x, np.float32)
    in_maps = make_in_maps(np.asarray(x, np.float32), np.asarray(rope_cos, np.float32),
                           np.asarray(rope_sin, np.float32), np.asarray(qkv_w, np.float32),
                           np.asarray(proj_w, np.float32), np.asarray(q_norm_w, np.float32),
                           np.asarray(k_norm_w, np.float32))
    nc = build_nc()
    trace = bool(os.environ.get("BASS_KERNEL_TRACE"))
    try:
        res = run_bass_kernel_spmd(nc, in_maps, list(range(NCORES)), trace=trace)
    except Exception:
        # transient device wedge (e.g. NRT_EXEC_UNIT_UNRECOVERABLE) — retry once
        res = run_bass_kernel_spmd(build_nc(), in_maps, list(range(NCORES)), trace=trace)
    LAST_RESULTS = res
    return gather(res.results, np.asarray(proj_b, np.float32))


if __name__ == "__main__":
    rng = np.random.default_rng(0)
    out = kernel(
        x=rng.standard_normal((B, N, C)).astype(np.float32),
        rope_cos=rng.random((N, D), dtype=np.float32),
        rope_sin=rng.random((N, D), dtype=np.float32),
        qkv_w=(rng.standard_normal((3 * C, C)) * C ** -0.5).astype(np.float32),
        proj_w=(rng.standard_normal((C, C)) * C ** -0.5).astype(np.float32),
        proj_b=np.zeros((C,), np.float32),
        q_norm_w=np.ones((D,), np.float32),
        k_norm_w=np.ones((D,), np.float32),
    )
    print(out.shape, out.dtype)

